# revision 12
# baseline (speedup 1.0000x reference)
"""Multi-head attention (8 heads, d_model=384) on 8 trn2 NeuronCores.

Sharding: data-parallel over batch (B=8 -> one batch element per core).

Per-core device kernel (all f32):
  - inputs are host-side transposed+augmented: xT_aug [512, S] holds x.T in
    rows 0:384, ones in row 384 (bias fusion), zeros elsewhere; weights are
    padded head-major [512, 512]: head h occupies output cols h*64..h*64+48
    (softmax scale folded into Wq; Wv additionally carries a fused ones
    column at h*64+48 so P@V also produces softmax denominators).
  - projections give qT/kT as [d_out_pad, S] tiles (heads at 64-row offsets
    inside 128-partition tiles) and v as natural [S, d_out_pad] tiles.
  - scores are computed transposed per head: S^T[sk, sq] = kT.T-slice @ qT,
    so softmax runs along partitions and P@V consumes P^T directly with v
    stationary (no transposes anywhere on the hot path).
  - exp on ACT straight out of PSUM; P^T@... accumulation in PSUM over the
    16 sk stripes; row 48 of the result is the softmax denominator.
  - denominator row is broadcast to 128 partitions with a ones[1,128]
    matmul, reciprocal on DVE, then stripes are normalized (DVE/GPSIMD) and
    DMA'd to attnT[h, sk, sq]. Host returns a transposed view.
"""
import sys

sys.path.insert(0, "/opt/trn_rl_repo")

import numpy as np

import concourse.bass as bass
import concourse.mybir as mybir
import concourse.tile as tile
from concourse import bacc
from concourse.bass_utils import run_bass_kernel_spmd

F32 = mybir.dt.float32
F32R = mybir.dt.float32r
F16 = mybir.dt.float16

B = 8
D = 384
H = 8
DH = 48
HP = 64          # padded per-head width in the q/k head-major layout
DP = H * HP      # 512: padded d_out for q/k
HPV = 64         # per-head stride in the v layout (ones col at +48)
DPV = H * HPV    # 512: padded d_out for v
DIN = 512        # augmented+padded d_in (384 data + 1 ones + pad)
KC = DIN // 128  # k-chunks for projections


def build_nc(S: int, BQ: int, stripe_bufs: int = 40):
    """Build the per-core Bass program. S = sequence length, BQ = sq block."""
    NSK = S // 128           # sk stripes
    NB = S // BQ             # sq blocks
    NQC = BQ // 512          # 512-wide matmul chunks per block
    assert BQ % 512 == 0 and S % BQ == 0 and S % 512 == 0

    nc = bacc.Bacc("TRN2", target_bir_lowering=False, debug=False)
    xq = nc.dram_tensor("xq", [DIN, S], F32R, kind="ExternalInput")
    xk = nc.dram_tensor("xk", [DIN, S], F32R, kind="ExternalInput")
    xv = nc.dram_tensor("xv", [DIN, S], F32R, kind="ExternalInput")
    wq = nc.dram_tensor("wq", [DIN, DP], F32R, kind="ExternalInput")
    wk = nc.dram_tensor("wk", [DIN, DP], F32R, kind="ExternalInput")
    wv = nc.dram_tensor("wv", [DIN, DPV], F32R, kind="ExternalInput")
    attnT = nc.dram_tensor("attnT", [H, S, S], F16, kind="ExternalOutput")
    ctxT = nc.dram_tensor("ctxT", [D, S], F32, kind="ExternalOutput")

    with tile.TileContext(nc) as tc:
        with tc.tile_pool(name="persist", bufs=1) as persist:
            qT = [persist.tile([128, S], F32R, tag=f"qT{i}", name=f"qT{i}") for i in range(4)]
            kT = [persist.tile([128, S], F32R, tag=f"kT{i}", name=f"kT{i}") for i in range(4)]
            vp = [persist.tile([128, DPV], F16, tag=f"vp{i}", name=f"vp{i}") for i in range(S // 128)]

            # ---- projections ----
            with (
                tc.tile_pool(name="xin", bufs=2 * KC + 1) as xpool,
                tc.tile_pool(name="wpool", bufs=1) as wpool,
                tc.tile_pool(name="pj", bufs=3, space="PSUM") as pjpool,
            ):
                w_sb = {}
                for name, wdram in (("q", wq), ("k", wk), ("v", wv)):
                    wid = DPV if name == "v" else DP
                    w_sb[name] = [
                        wpool.tile([128, wid], F32R, tag=f"w{name}{i}", name=f"w{name}{i}") for i in range(KC)
                    ]
                    for i in range(KC):
                        nc.sync.dma_start(
                            out=w_sb[name][i], in_=wdram[i * 128:(i + 1) * 128, :]
                        )

                # qT / kT: weights stationary, activations streamed
                for name, xdram, outT in (("q", xq, qT), ("k", xk, kT)):
                    xt = [xpool.tile([128, S], F32R, tag="x", name="x") for _ in range(KC)]
                    for i in range(KC):
                        nc.sync.dma_start(out=xt[i], in_=xdram[i * 128:(i + 1) * 128, :])
                    for m in range(4):
                        for n in range(S // 512):
                            ps = pjpool.tile([128, 512], F32, tag="pj")
                            for kc in range(KC):
                                nc.tensor.matmul(
                                    ps,
                                    lhsT=w_sb[name][kc][:, m * 128:(m + 1) * 128],
                                    rhs=xt[kc][:, n * 512:(n + 1) * 512],
                                    start=(kc == 0),
                                    stop=(kc == KC - 1),
                                )
                            nc.vector.tensor_copy(outT[m][:, n * 512:(n + 1) * 512], ps)

                # v: activations stationary, weights streamed -> natural layout
                xt = [xpool.tile([128, S], F32R, tag="x", name="x") for _ in range(KC)]
                for i in range(KC):
                    nc.sync.dma_start(out=xt[i], in_=xv[i * 128:(i + 1) * 128, :])
                for mt in range(S // 128):
                    ps = pjpool.tile([128, DPV], F32, tag="pj", name="psv")
                    for kc in range(KC):
                        nc.tensor.matmul(
                            ps,
                            lhsT=xt[kc][:, mt * 128:(mt + 1) * 128],
                            rhs=w_sb["v"][kc],
                            start=(kc == 0),
                            stop=(kc == KC - 1),
                        )
                    nc.vector.tensor_copy(vp[mt], ps)

            # ---- attention heads ----
            with (
                tc.tile_pool(name="stripes", bufs=stripe_bufs) as stp,
                tc.tile_pool(name="misc", bufs=2) as misc,
                tc.tile_pool(name="dscratch", bufs=4, space="DRAM") as dsc_pool,
                tc.tile_pool(name="spsum", bufs=3, space="PSUM") as sps_pool,
                tc.tile_pool(name="cpsum", bufs=1, space="PSUM") as cps_pool,
            ):
                # heads processed in even/odd pairs: their score matmuls use
                # base partitions 0 and 64 -> distinct PE row-groups -> the
                # two matmuls run concurrently in the array
                for hp in range(H // 2):
                    t = hp
                    heads = (2 * hp, 2 * hp + 1)
                    for blk in range(NB):
                        q0 = blk * BQ
                        cps = cps_pool.tile([128, BQ], F32, tag="c", name="cps")
                        pts = {0: [], 1: []}
                        for j in range(NSK):
                            spss = [
                                sps_pool.tile([128, BQ], F32, tag="s", name="sps")
                                for _ in heads
                            ]
                            for n in range(NQC):
                                for hi in range(2):
                                    off = hi * 64
                                    nc.tensor.matmul(
                                        spss[hi][:, n * 512:(n + 1) * 512],
                                        lhsT=kT[t][off:off + DH, j * 128:(j + 1) * 128],
                                        rhs=qT[t][off:off + DH, q0 + n * 512:q0 + (n + 1) * 512],
                                        start=True,
                                        stop=True,
                                    )
                            for hi, h in enumerate(heads):
                                pt = stp.tile([128, BQ], F16, tag="pt")
                                nc.scalar.activation(
                                    pt, spss[hi], mybir.ActivationFunctionType.Exp
                                )
                                for n in range(NQC):
                                    nc.tensor.matmul(
                                        cps[hi * 64:hi * 64 + DH + 1, n * 512:(n + 1) * 512],
                                        lhsT=vp[j][:, h * HPV:h * HPV + DH + 1],
                                        rhs=pt[:, n * 512:(n + 1) * 512],
                                        start=(j == 0),
                                        stop=(j == NSK - 1),
                                        tile_position=(0, hi * 64),
                                    )
                                pts[hi].append(pt)
                        for hi, h in enumerate(heads):
                            # C' block for this head: rows 0:48 = context
                            # numerators, row 48 = softmax denominators
                            t64 = misc.tile([64, BQ], F32, tag="t64")
                            nc.vector.tensor_copy(t64, cps[hi * 64:(hi + 1) * 64, :])
                            dsq = misc.tile([128, BQ // 128], F32, tag="dsq")
                            nc.sync.dma_start(out=dsq, in_=t64[DH:DH + 1, :])
                            rsq = misc.tile([128, BQ // 128], F16, tag="rsq")
                            with nc.allow_low_precision(
                                reason="f16 softmax denominators are plenty"
                            ):
                                nc.vector.reciprocal(rsq, dsq)
                            dscr = dsc_pool.tile([BQ], F16, tag="dscr")
                            nc.sync.dma_start(out=dscr, in_=rsq)
                            rb16 = misc.tile([128, BQ], F16, tag="rb16")
                            nc.sync.dma_start(
                                out=rb16,
                                in_=bass.AP(
                                    tensor=dscr.tensor,
                                    offset=dscr.offset,
                                    ap=[[0, 128]] + list(dscr.ap),
                                ),
                            )
                            # context slice for this (head, block)
                            ctx = misc.tile([DH, BQ], F32, tag="ctx")
                            nc.vector.tensor_mul(ctx, t64[0:DH, :], rb16[0:DH, :])
                            nc.sync.dma_start(
                                out=ctxT[h * DH:(h + 1) * DH, q0:q0 + BQ], in_=ctx
                            )
                            # normalize + write out the attention stripes
                            for j in range(NSK):
                                nc.vector.tensor_mul(pts[hi][j], pts[hi][j], rb16)
                                nc.sync.dma_start(
                                    out=attnT[h, j * 128:(j + 1) * 128, q0:q0 + BQ],
                                    in_=pts[hi][j],
                                )
    nc.finalize()
    return nc


def _prep_weights(Wq, bq, Wk, bk, Wv, bv):
    """Padded head-major, bias-augmented weight mats [DIN, DP]."""
    scale = np.float32(1.0 / np.sqrt(np.float32(DH)))
    wqp = np.zeros((DIN, DP), np.float32)
    wkp = np.zeros((DIN, DP), np.float32)
    wvp = np.zeros((DIN, DPV), np.float32)
    for h in range(H):
        src = slice(h * DH, (h + 1) * DH)
        dst = slice(h * HP, h * HP + DH)
        dstv = slice(h * HPV, h * HPV + DH)
        wqp[0:D, dst] = Wq[:, src] * scale
        wqp[D, dst] = bq[src] * scale
        wkp[0:D, dst] = Wk[:, src]
        wkp[D, dst] = bk[src]
        wvp[0:D, dstv] = Wv[:, src]
        wvp[D, dstv] = bv[src]
        wvp[D, h * HPV + DH] = 1.0  # fused ones column -> softmax denominators
    return wqp, wkp, wvp


def _prep_x(x):
    """[S, D] -> transposed+augmented [DIN, S]."""
    S = x.shape[0]
    xa = np.zeros((DIN, S), np.float32)
    xa[0:D] = x.T
    xa[D] = 1.0
    return xa


def run_attention(query, key, value, Wq, bq, Wk, bk, Wv, bv, S, BQ, trace=False,
                  **run_kwargs):
    nb = query.shape[0]
    nc = build_nc(S, BQ)
    wqp, wkp, wvp = _prep_weights(Wq, bq, Wk, bk, Wv, bv)
    in_maps = []
    for b in range(nb):
        in_maps.append(
            {
                "xq": _prep_x(query[b]),
                "xk": _prep_x(key[b]),
                "xv": _prep_x(value[b]),
                "wq": wqp,
                "wk": wkp,
                "wv": wvp,
            }
        )
    res = run_bass_kernel_spmd(
        nc, in_maps, core_ids=list(range(nb)), trace=trace, **run_kwargs
    )
    # gather: context[b] = ctxT_b.T ; attn[h*B+b, sq, sk] = attnT_b[h, sk, sq]
    context = np.stack([res.results[b]["ctxT"].T for b in range(nb)], axis=0)
    attn = (
        np.stack([res.results[b]["attnT"] for b in range(nb)], axis=1)
        .astype(np.float32)
        .reshape(H * nb, S, S)
        .swapaxes(1, 2)
    )
    return (context, attn), res


def kernel(query, key, value, Wq, bq, Wk, bk, Wv, bv):
    query = np.asarray(query, np.float32)
    key = np.asarray(key, np.float32)
    value = np.asarray(value, np.float32)
    Wq = np.asarray(Wq, np.float32)
    bq = np.asarray(bq, np.float32)
    Wk = np.asarray(Wk, np.float32)
    bk = np.asarray(bk, np.float32)
    Wv = np.asarray(Wv, np.float32)
    bv = np.asarray(bv, np.float32)
    S = query.shape[1]
    out, _ = run_attention(
        query, key, value, Wq, bq, Wk, bk, Wv, bv, S=S, BQ=1024
    )
    return out
